# revision 53
# baseline (speedup 1.0000x reference)
"""Single-head attention (B=4, N=2048, D=1024) on 8 Trainium2 NeuronCores.

Sharding: core c handles batch c//2 and KEY half c%2, with the duplicated
Q projection eliminated by a pairwise AllGather.  Each core receives only
its key-half of x (xTk, 2MB) plus the three weights; it computes K/V
projections and Q for its OWN 1024 queries (= its key half), then
AllGathers the pair's qT shards through DRAM bounce buffers while the
remaining K/V projection work hides the collective's ~27us fixed latency.
Scores/AV then run over its 1024 keys x all 2048 queries (global order),
producing the partial (unnormalized) attention output and partial softmax
denominator.  The host combines halves: out = (oA + oB) / (dA + dB).

Precision: projections/AV in bf16 (fp32 PSUM accumulation).  The scores
contraction is split: e-blocks 0-3 bf16, e-blocks 4-7 fp8e4 DoubleRow
(two contraction rows per PE cell, 2x bf16 throughput measured).  This
half-fp8 split measures rel err ~1.25e-2 against the f32 reference (full
fp8 was 1.89e-2 — too close to the 2e-2 gate).  exp in fp32 on the
scalar engine; unnormalized softmax (no max subtraction) is safe since
|scores/sqrt(D)| is ~N(0, 0.33^2).  Partial outputs are stored bf16.
"""

from contextlib import ExitStack

import ml_dtypes
import numpy as np

import concourse.bass as bass
import concourse.mybir as mybir
import concourse.tile as tile
from concourse.bass_utils import run_bass_kernel_spmd

B, N, D = 4, 2048, 1024
NCORES = 8
P = 128
NQ = N            # total queries per batch (gathered)
NKH = N // 2      # keys (and local queries) per core
DC = D // P       # 8 contraction chunks
EC = D // P       # 8 embed blocks
JB = NKH // P     # 8 key blocks
F = 512           # matmul moving free dim (one PSUM bank of fp32)
SCALE = 1.0 / np.sqrt(D)
N_WARM_PRE = 2    # dummy matmuls bridging trigger latency until chunk 0
                  # lands; the chunk-gated loop itself (cold-rate work >
                  # arrival pace) then sustains the HAM warm-up window
NBF = 2           # scores e-blocks 0..NBF-1 in bf16; the rest fp8 DoubleRow

BF = mybir.dt.bfloat16
F8 = mybir.dt.float8e4
F32 = mybir.dt.float32

REPLICA_GROUPS = [[0, 1], [2, 3], [4, 5], [6, 7]]

QBF_B = NBF * NKH * 2                 # bytes of the bf16 qT/kT half: 8192
QRANK_B = QBF_B + (EC - NBF) * NKH    # bytes per rank shard: 12288


def _attention_kernel(ctx, tc, out, xTk, wqT, wkT, wvT):
    nc = tc.nc

    consts = ctx.enter_context(tc.tile_pool(name="consts", bufs=1))
    psmain = ctx.enter_context(tc.tile_pool(name="psmain", bufs=2, space="PSUM"))
    psav = ctx.enter_context(tc.tile_pool(name="psav", bufs=6, space="PSUM"))
    outp_big = ctx.enter_context(tc.tile_pool(name="outp_big", bufs=1))
    outp_sm = ctx.enter_context(tc.tile_pool(name="outp_sm", bufs=3))
    small = ctx.enter_context(tc.tile_pool(name="small", bufs=2))
    dram = ctx.enter_context(tc.tile_pool(name="dram", bufs=1, space="DRAM"))

    # Resident SBUF tensors.  qT/kT/qTloc are byte-granular tiles holding
    # a bf16 half (e-blocks 0-3) and an fp8 half (e-blocks 4-7) exposed
    # through bitcast views, so every gather hop is ONE DMA.  qT is
    # rank-major: rank r's shard is a contiguous per-partition byte range.
    xTk_sb = consts.tile([P, DC, NKH], BF, tag="xTk")    # [p, d-chunk, key]
    wkv_sb = consts.tile([P, 2 * DC * D], BF, tag="wkv")
    wk_sb = wkv_sb.rearrange("p (two c e) -> p two c e", two=2, c=DC)[:, 0]
    wv_sb = wkv_sb.rearrange("p (two c e) -> p two c e", two=2, c=DC)[:, 1]
    wq_sb = consts.tile([P, DC, D], BF, tag="wq")
    qT_sb = consts.tile([P, 2, QRANK_B], F8, tag="qT")
    qTloc_sb = consts.tile([P, QRANK_B], F8, tag="qTloc")
    kT_sb = consts.tile([P, QRANK_B], F8, tag="kT")
    v_sb = consts.tile([P, JB, D], BF, tag="v")          # [p, key-block, e]
    pT_sb = consts.tile([P, JB, NQ], BF, tag="pT")       # [p, key-block, query]
    ones_sb = consts.tile([P, 1], BF, tag="ones")

    def _views(t):  # byte range -> (bf16 [P,NBF,NKH], fp8 [P,EC-NBF,NKH])
        bf = t[:, 0:QBF_B].bitcast(BF).rearrange("p (e j) -> p e j", e=NBF)
        f8 = t[:, QBF_B:QRANK_B].rearrange("p (e j) -> p e j", e=EC - NBF)
        return bf, f8

    qTloc_bf, qTloc_f8 = _views(qTloc_sb)
    kT_bf, kT_f8 = _views(kT_sb)
    qT_rk = [_views(qT_sb[:, r, :]) for r in range(2)]

    # DRAM bounce buffers for the pairwise qT AllGather (mixed payload:
    # 1.5MB out, 3MB back).
    cc_in = dram.tile([P, QRANK_B], F8, name="cc_in")
    cc_out = dram.tile([2, P, QRANK_B], F8, name="cc_out")

    nc.vector.memset(ones_sb, 1.0)

    xTr = xTk.rearrange("(c p) j -> p c j", p=P)
    wqr = wqT.rearrange("(c p) e -> p c e", p=P)
    wkr = wkT.rearrange("(c p) e -> p c e", p=P)
    wvr = wvT.rearrange("(c p) e -> p c e", p=P)

    # Input DMAs.  The per-core HBM read port (~358 GB/s) is the early
    # bottleneck: 8MB of input takes ~22us to land.  Phase 1a needs
    # wk + xTk (4MB) chunk-by-chunk ASAP, then wq chunks feed phase
    # 2a-local; wv (needed last, ~60us) follows as one large DMA.
    # Per-queue FIFO on the HWDGE queues preserves this priority.
    xk_dmas = []
    wk_dmas = []
    for c in range(DC):
        xk_dmas.append(nc.sync.dma_start(out=xTk_sb[:, c, :], in_=xTr[:, c, :]))
        wk_dmas.append(nc.sync.dma_start(out=wk_sb[:, c, :], in_=wkr[:, c, :]))
    wq_dmas = []
    for c in range(DC):
        wq_dmas.append(nc.sync.dma_start(out=wq_sb[:, c, :], in_=wqr[:, c, :]))
    wv_dma = nc.sync.dma_start(out=wv_sb[:, :, :], in_=wvr)
    in_dmas = xk_dmas + wk_dmas + wq_dmas + [wv_dma]

    def sp_observe(inst, why):
        n = nc.sync.nop(hint="observe")
        tile.add_dep_helper(n.ins, inst.ins, reason=why)

    # One PSUM tile for dummy warm-up and touch matmuls.  It comes from the
    # psav pool, which no DVE copy reads until phase 2c — so every write to
    # it is PE-local and touch matmuls carry exactly one (DMA) wait.
    warm_src = small.tile([P, 640], BF, tag="warm")
    nc.vector.memset(warm_src, 0.0)
    warm_ps = psav.tile([P, F], F32, tag="po")

    def dummy():
        nc.tensor.matmul(
            warm_ps, lhsT=warm_src[:, 0:P], rhs=warm_src[:, P : P + F],
            start=True, stop=True,
        )

    def touch(t):
        # Trivial matmul whose only purpose is to make the PE observe t's
        # producer (single sync wait), so later real matmuls need none.
        nc.tensor.matmul(
            warm_ps[0:1, 0:1], lhsT=t[:, 0:1], rhs=t[:, 0:1], start=True, stop=True
        )

    # Solid warm-up block: HAM un-throttles only after a ~3.4us window of
    # SUSTAINED PE activity; scattered chunk-gated matmuls never produce one.
    for _ in range(N_WARM_PRE):
        dummy()

    def kT_out(e, jt):
        if e < NBF:
            return kT_bf[:, e, jt * F : (jt + 1) * F]
        return kT_f8[:, e - NBF, jt * F : (jt + 1) * F]

    # Phase 1a e=0: kT[0, j] — chunk-major, gated on each (xk, wk) chunk
    # pair as it lands (~1.4us apart at HBM rate), with touch matmuls
    # carrying the DMA waits and interleaved dummies keeping the PE duty
    # high so HAM stays warm.
    ps0 = psmain.tile([P, F], F32, tag="ps")
    ps1 = psmain.tile([P, F], F32, tag="ps")
    for c in range(DC):
        touch(xTk_sb[:, c, :])
        touch(wk_sb[:, c, :])
        nc.tensor.matmul(
            ps0, lhsT=wk_sb[:, c, 0:P], rhs=xTk_sb[:, c, 0:F],
            start=(c == 0), stop=(c == DC - 1),
        )
        nc.tensor.matmul(
            ps1, lhsT=wk_sb[:, c, 0:P], rhs=xTk_sb[:, c, F : 2 * F],
            start=(c == 0), stop=(c == DC - 1),
        )
        dummy()
        dummy()
    nc.vector.tensor_copy(out=kT_out(0, 0), in_=ps0)
    nc.vector.tensor_copy(out=kT_out(0, 1), in_=ps1)

    # Phase 2a-local RIGHT AFTER the gated block: qT[e, j_local] for this
    # core's OWN 1024 queries, so the AllGather chain launches ~50us in.
    # Its first groups gate on the wq chunk stream (landing ~20-26us);
    # interleaved dummies keep the duty high.  Only wq chunk 0 needs a
    # touch: each group's START matmul carries the PSUM-reuse wait, so it
    # must not also wait on a DMA; later chunks' waits ride legally on the
    # non-start matmuls (one wait each).
    touch(wq_sb[:, 0, :])
    for e in range(EC):
        for it in range(NKH // F):
            ps = psmain.tile([P, F], F32, tag="ps")
            for c in range(DC):
                nc.tensor.matmul(
                    ps,
                    lhsT=wq_sb[:, c, e * P : (e + 1) * P],
                    rhs=xTk_sb[:, c, it * F : (it + 1) * F],
                    start=(c == 0),
                    stop=(c == DC - 1),
                )
                if e == 0:
                    dummy()
            qdst = (
                qTloc_bf[:, e, it * F : (it + 1) * F]
                if e < NBF
                else qTloc_f8[:, e - NBF, it * F : (it + 1) * F]
            )
            nc.vector.tensor_copy(out=qdst, in_=ps)
    # Ship the local shard and AllGather across the core pair.  All bounce
    # traffic rides gpsimd's SWDGE queues (3 + 4 output stores = 7 DMAs,
    # at most one per queue — no queue-lap waits).  One DMA per hop so
    # each instruction carries a single sync wait (the collective cannot
    # aggregate multiple input-piece semaphores).
    cc_in_dma = nc.gpsimd.dma_start(out=cc_in[:, :], in_=qTloc_sb[:, :])
    cc = nc.gpsimd.collective_compute(
        "AllGather",
        mybir.AluOpType.bypass,
        replica_groups=REPLICA_GROUPS,
        ins=[cc_in[:, :].opt()],
        outs=[cc_out[:, :, :].opt()],
    )

    # Phase 1a remainder (kT e-blocks 1-7) + wv touch (wv lands ~31us; the
    # PE reaches e=4 ~62us).
    def kT_block(e):
        for jt in range(NKH // F):
            ps = psmain.tile([P, F], F32, tag="ps")
            for c in range(DC):
                nc.tensor.matmul(
                    ps,
                    lhsT=wk_sb[:, c, e * P : (e + 1) * P],
                    rhs=xTk_sb[:, c, jt * F : (jt + 1) * F],
                    start=(c == 0),
                    stop=(c == DC - 1),
                )
            nc.vector.tensor_copy(out=kT_out(e, jt), in_=ps)

    for e in range(1, EC):
        kT_block(e)
        if e == 4:
            touch(wv_sb[:, 0, :])

    # Phase 1b: v[j, e] — lhsT = xTk[d, j-blk], rhs = WvT[d, e-tile]
    for j in range(JB):
        for et in range(D // F):
            ps = psmain.tile([P, F], F32, tag="ps")
            for c in range(DC):
                nc.tensor.matmul(
                    ps,
                    lhsT=xTk_sb[:, c, j * P : (j + 1) * P],
                    rhs=wv_sb[:, c, et * F : (et + 1) * F],
                    start=(c == 0),
                    stop=(c == DC - 1),
                )
            nc.vector.tensor_copy(out=v_sb[:, j, et * F : (et + 1) * F], in_=ps)

    # Read the gathered qT shards back, one DMA per rank, STAGGERED (the
    # nop serializes rank 1 behind rank 0) so rank 0 gets full read
    # bandwidth and phase 2b can start on its query tiles ~2us sooner.
    # qT_sb is fresh, so each read-back's only dependency is the previous
    # hop — exactly one wait per DMA.
    qt_rb0 = nc.gpsimd.dma_start(out=qT_sb[:, 0, :], in_=cc_out[0])
    n_rb = nc.gpsimd.nop(hint="observe")
    tile.add_dep_helper(n_rb.ins, qt_rb0.ins, reason="stagger rank-1 read-back")
    qt_rb1 = nc.gpsimd.dma_start(out=qT_sb[:, 1, :], in_=cc_out[1])

    # Phase 2b: scoresT[j, i] = k @ q.T over this key half, p = exp(s*SCALE).
    # Mixed contraction: e-blocks 0-3 bf16, e-blocks 4-7 as two fp8
    # DoubleRow matmuls (3D APs [128, 2, n]; middle dim = the interleaved
    # contraction-row pair), accumulating into one PSUM group.  Tiles are
    # processed rank-major (all rank-0 query tiles first) so compute can
    # begin as soon as rank 0's read-back lands; within a rank the
    # bf16/DoubleRow order snakes so consecutive tiles share the PE
    # weight-path mode at the boundary (mode switches cost ~200ns).
    def scores_tile(j, rk, itr, flip):
        q_bf, q_f8 = qT_rk[rk]
        ps = psmain.tile([P, F], F32, tag="ps")
        bf_mms = [
            dict(
                lhsT=kT_bf[:, e, j * P : (j + 1) * P],
                rhs=q_bf[:, e, itr * F : (itr + 1) * F],
                perf_mode=None,
            )
            for e in range(NBF)
        ]
        f8_mms = [
            dict(
                lhsT=kT_f8[:, e : e + 2, j * P : (j + 1) * P],
                rhs=q_f8[:, e : e + 2, itr * F : (itr + 1) * F],
                perf_mode=mybir.MatmulPerfMode.DoubleRow,
            )
            for e in range(0, EC - NBF, 2)
        ]
        mms = bf_mms + f8_mms if not flip else f8_mms + bf_mms
        for i, kw in enumerate(mms):
            nc.tensor.matmul(
                ps,
                lhsT=kw["lhsT"],
                rhs=kw["rhs"],
                start=(i == 0),
                stop=(i == len(mms) - 1),
                perf_mode=kw["perf_mode"],
            )
        return ps

    flip = False
    last_exp = None
    for rk in range(2):
        # Absorb this rank's read-back DMA wait on the PE.
        touch(qT_sb[:, rk, 0:1])
        for j in range(JB):
            for itr in range(NKH // F):
                it = rk * (NKH // F) + itr
                ps = scores_tile(j, rk, itr, flip)
                flip = not flip
                last_exp = nc.scalar.activation(
                    out=pT_sb[:, j, it * F : (it + 1) * F],
                    in_=ps,
                    func=mybir.ActivationFunctionType.Exp,
                    scale=float(SCALE),
                )

    for dmad in in_dmas:
        sp_observe(dmad, "observe input DMA on SP")
    sp_observe(cc_in_dma, "observe cc bounce-in DMA on SP")

    # Phase 2c: partial out[i, 0:1024] = pT.T @ v, partial denom in column
    # 1024 (folded into the same output tensor).  FOUR stores sized
    # {5,1,1,1} query-block groups: the big store issues mid-phase when
    # write bandwidth is free, the three small ones trickle out ~3.6us
    # apart, so the end-of-kernel drain only covers 525KB.  Every store
    # has its own buffer — no WAR guards needed.  Stored bf16.
    outr = out.rearrange("(gg p) e -> p gg e", p=P)   # [P, 16, D+1]
    STORES = [(0, 5), (5, 1), (6, 1), (7, 1)]         # (start ib2, n ib2)
    out_dmas = []
    for s, (start, ng) in enumerate(STORES):
        pool = outp_big if ng > 1 else outp_sm
        o_sb = pool.tile([P, 2 * ng, D + 1], BF, tag="o")
        g2 = nc.vector.memset(o_sb[0:1, 0, 0:1], 0.0)
        for gi in range(ng):
            ib2 = start + gi
            for t in range(2):
                ib = 2 * ib2 + t
                tl = 2 * gi + t
                po0 = psav.tile([P, F], F32, tag="po")
                po1 = psav.tile([P, F], F32, tag="po")
                pd = psav.tile([P, F], F32, tag="po")
                for j in range(JB):
                    lhsT = pT_sb[:, j, ib * P : (ib + 1) * P]
                    nc.tensor.matmul(
                        po0, lhsT=lhsT, rhs=v_sb[:, j, 0:F],
                        start=(j == 0), stop=(j == JB - 1),
                    )
                    nc.tensor.matmul(
                        po1, lhsT=lhsT, rhs=v_sb[:, j, F : 2 * F],
                        start=(j == 0), stop=(j == JB - 1),
                    )
                    last_mm = nc.tensor.matmul(
                        pd[:, 0:1], lhsT=lhsT, rhs=ones_sb,
                        start=(j == 0), stop=(j == JB - 1),
                    )
                # Denominator copy first: pd's stop-matmul is the group's
                # last PE tick, so this copy's PE wait covers po0/po1 and
                # the po copies need only their (buffer-reuse) DVE wait.
                dcp = nc.vector.tensor_copy(
                    out=o_sb[:, tl, D : D + 1], in_=pd[:, 0:1]
                )
                tile.add_dep_helper(
                    dcp.ins, g2.ins, False, reason="order after guard"
                )
                c0 = nc.vector.tensor_copy(out=o_sb[:, tl, 0:F], in_=po0)
                tile.add_dep_helper(c0.ins, dcp.ins, False, reason="order after dcp")
                last_cp = nc.vector.tensor_copy(out=o_sb[:, tl, F : 2 * F], in_=po1)
                tile.add_dep_helper(last_cp.ins, c0.ins, False, reason="order after c0")
        out_dmas.append(
            nc.gpsimd.dma_start(
                out=outr[:, 2 * start : 2 * (start + ng), :], in_=o_sb
            )
        )

    for dd in out_dmas:
        sp_observe(dd, "observe output DMA on SP")
    sp_observe(qt_rb0, "observe qT read-back 0 on SP")
    sp_observe(qt_rb1, "observe qT read-back 1 on SP")
    sp_observe(last_exp, "observe ACT on SP")
    sp_observe(last_mm, "observe PE on SP")
    sp_observe(last_cp, "observe DVE on SP")


def build_attention_module():
    nc = bass.Bass(trn_type="TRN2", target_bir_lowering=False, debug=False)
    xTk = nc.dram_tensor("xTk", [D, NKH], BF, kind="ExternalInput").ap()
    wqT = nc.dram_tensor("wqT", [D, D], BF, kind="ExternalInput").ap()
    wkT = nc.dram_tensor("wkT", [D, D], BF, kind="ExternalInput").ap()
    wvT = nc.dram_tensor("wvT", [D, D], BF, kind="ExternalInput").ap()
    out = nc.dram_tensor("out", [NQ, D + 1], BF, kind="ExternalOutput").ap()
    with tile.TileContext(nc) as tc:
        with ExitStack() as ctx:
            _attention_kernel(ctx, tc, out, xTk, wqT, wkT, wvT)
    return nc


_module_cache = None


def _get_module():
    global _module_cache
    if _module_cache is None:
        _module_cache = build_attention_module()
    return _module_cache


def make_in_maps(x, Wq, Wk, Wv):
    bf = ml_dtypes.bfloat16
    x = np.asarray(x, dtype=np.float32)
    wq = np.asarray(Wq, dtype=np.float32).T.astype(bf)
    wk = np.asarray(Wk, dtype=np.float32).T.astype(bf)
    wv = np.asarray(Wv, dtype=np.float32).T.astype(bf)
    in_maps = []
    for core in range(NCORES):
        b, half = divmod(core, 2)
        xtk = x[b].T[:, half * NKH : (half + 1) * NKH]  # [D, NKH]
        in_maps.append(
            {
                "xTk": np.ascontiguousarray(xtk).astype(bf),
                "wqT": wq,
                "wkT": wk,
                "wvT": wv,
            }
        )
    return in_maps


def _install_ntff_hook_shim():
    """The container's `antenv` stub lacks axon_hooks; register an equivalent
    built on trn_agent_boot's ctypes NTFF driver so trace=True works."""
    import sys
    import types

    if "antenv.axon_hooks" in sys.modules:
        return
    try:
        from trn_agent_boot.trn_boot import _ntff_profile_via_ctypes

        hook = _ntff_profile_via_ctypes("/opt/axon/libaxon_pjrt.so")
    except Exception:
        hook = None
    mod = types.ModuleType("antenv.axon_hooks")
    mod.get_axon_ntff_profile_hook = lambda: hook
    sys.modules["antenv.axon_hooks"] = mod


def kernel(x, Wq, Wk, Wv, _trace=False, _trace_cores=None):
    if _trace:
        _install_ntff_hook_shim()
    in_maps = make_in_maps(x, Wq, Wk, Wv)
    nc = _get_module()
    res = run_bass_kernel_spmd(
        nc,
        in_maps,
        core_ids=list(range(NCORES)),
        trace=_trace,
        trace_cores=_trace_cores,
    )
    out = np.empty((B, N, D), dtype=np.float32)
    for b in range(B):
        r0 = res.results[2 * b]["out"].astype(np.float32)
        r1 = res.results[2 * b + 1]["out"].astype(np.float32)
        osum = r0 + r1
        out[b] = osum[:, :D] / osum[:, D : D + 1]
    if _trace:
        return out, res
    return out
